# revision 7
# baseline (speedup 1.0000x reference)
"""GAT (3-layer, PyG-style) forward on 8 Trainium2 NeuronCores via Bass/Tile.

Strategy (per core, SPMD):
  - Nodes are padded to NP=50176 and dst-sharded: core c owns nodes
    [c*6272, (c+1)*6272) = 49 blocks of 128.
  - Per layer: every core produces the full "table" hw_aug = h @ W_aug
    (W_aug = [W | W.a_src | W.a_dst] folded) into its local HBM in bf16,
    then gathers, per dst-block, the table rows of its edges' sources
    with the dma_gather custom DMA instruction (int16 indices; sources
    split into two gathers around row 32768 to fit int16).
  - Per 128-dst block: edge weights w = exp(leaky_relu(asrc_src + adst_dst))
    are computed from gathered columns; messages are scaled by w and
    segment-summed into the 128 dst rows with one-hot selection matmuls
    on the tensor engine (S01[e, d] = (dst_local(e) == d)).
  - Softmax denominators ride along as extra matmul columns; epilogue
    normalizes, head-means, adds bias, applies ELU, transposes h for the
    next layer's produce matmul, and an AllGather shares h across cores.
  - Final pooling over graphs + the 3-layer MLP run on the host (tiny).
"""

import os
import sys
import types

import numpy as np
import ml_dtypes

import concourse.bass as bass
import concourse.bacc as bacc
import concourse.mybir as mybir
import concourse.tile as tile
from concourse.bass_utils import run_bass_kernel_spmd

BF16 = ml_dtypes.bfloat16

# Problem constants (nn_GAT_G_42760694399686)
N = 50000
E0 = 800000
F_IN = 128
HID = 64
H12 = 4
G = 256
NEG_SLOPE = 0.2

P = 128
CORES = 8
NP = 50176              # padded nodes: 8 * 49 * 128
NPC = NP // CORES       # 6272 nodes per core
BPC = NPC // P          # 49 blocks per core
NR = NP + 2             # table rows: sentinel-lo, NP nodes, sentinel-hi
SPLIT = 32768           # gather-B base row (int16 index limit)
SENT_LO = 0
SENT_HI = NR - 1
ASRC_SENT = -30000.0

LAST_EXEC_NS = None


def _install_ntff_shim():
    """antenv.axon_hooks is missing in this image; recreate it so
    run_bass_kernel_spmd(trace=True) can profile via the axon .so."""
    if 'antenv.axon_hooks' in sys.modules:
        return
    try:
        mod = types.ModuleType('antenv.axon_hooks')
        _hook = [None]
        mod.set_axon_ntff_profile_hook = lambda h: _hook.__setitem__(0, h)
        mod.get_axon_ntff_profile_hook = lambda: _hook[0]
        sys.modules['antenv.axon_hooks'] = mod
        import antenv
        antenv.axon_hooks = mod
        from trn_agent_boot.trn_boot import _ntff_profile_via_ctypes
        mod.set_axon_ntff_profile_hook(_ntff_profile_via_ctypes('/opt/axon/libaxon_pjrt.so'))
    except Exception:
        pass


# Layer configs: (in_feats, heads, table_cols, hw_cols, asrc_off, adst_off)
def _layer_cfgs():
    return [
        dict(F=F_IN, H=H12, TCH=384, HWC=H12 * HID, ASO=H12 * HID, ADO=H12 * HID + H12, OC=384),
        dict(F=HID, H=H12, TCH=384, HWC=H12 * HID, ASO=H12 * HID, ADO=H12 * HID + H12, OC=384),
        dict(F=HID, H=1, TCH=128, HWC=HID, ASO=HID, ADO=HID + 1, OC=128),
    ]


def build_program(TA, TB):
    """Build the SPMD Bass program. TA/TB: gather tile counts (per block)
    for the low/high source-row halves."""
    T = TA + TB
    NT = NP // P  # produce tiles per layer (392)
    dt = mybir.dt
    cfgs = _layer_cfgs()

    nc = bacc.Bacc("TRN2", target_bir_lowering=False, debug=True,
                   num_swdge_queues=4)

    xT = nc.declare_dram_parameter("xT", [P, NP], dt.bfloat16, isOutput=False)
    w1aug = nc.declare_dram_parameter("w1aug", [F_IN, cfgs[0]['OC']], dt.bfloat16, isOutput=False)
    w2aug = nc.declare_dram_parameter("w2aug", [HID, cfgs[1]['OC']], dt.bfloat16, isOutput=False)
    w3aug = nc.declare_dram_parameter("w3aug", [HID, cfgs[2]['OC']], dt.bfloat16, isOutput=False)
    sent12 = nc.declare_dram_parameter("sent12", [1, 384], dt.bfloat16, isOutput=False)
    sent3 = nc.declare_dram_parameter("sent3", [1, 128], dt.bfloat16, isOutput=False)
    bias1 = nc.declare_dram_parameter("bias1", [P, HID], dt.float32, isOutput=False)
    bias2 = nc.declare_dram_parameter("bias2", [P, HID], dt.float32, isOutput=False)
    bias3 = nc.declare_dram_parameter("bias3", [P, HID], dt.float32, isOutput=False)
    idx = nc.declare_dram_parameter("idx", [BPC, P, T * 8], dt.int16, isOutput=False)
    dstc = nc.declare_dram_parameter("dstc", [BPC, P, T], dt.bfloat16, isOutput=False)
    dstr = nc.declare_dram_parameter("dstr", [BPC, 1, T * P], dt.bfloat16, isOutput=False)
    adst1own = nc.declare_dram_parameter("adst1own", [BPC, P, H12], dt.bfloat16, isOutput=False)
    out3 = nc.declare_dram_parameter("out3", [NPC, HID], dt.float32, isOutput=True)

    with tile.TileContext(nc) as tc:
        with (
            tc.tile_pool(name="const", bufs=1) as cpool,
            tc.tile_pool(name="sb", bufs=2) as sb,
            tc.tile_pool(name="sb3", bufs=3) as sb3,
            tc.tile_pool(name="ps", bufs=2, space="PSUM") as ps,
            tc.tile_pool(name="dram", bufs=1, space="DRAM") as dram,
        ):
            # ---- constants ----
            w1aug_t = cpool.tile([F_IN, cfgs[0]['OC']], dt.bfloat16, tag="w1")
            nc.sync.dma_start(out=w1aug_t[:], in_=w1aug[:])
            w2aug_t = cpool.tile([HID, cfgs[1]['OC']], dt.bfloat16, tag="w2")
            nc.sync.dma_start(out=w2aug_t[:], in_=w2aug[:])
            w3aug_t = cpool.tile([HID, cfgs[2]['OC']], dt.bfloat16, tag="w3")
            nc.sync.dma_start(out=w3aug_t[:], in_=w3aug[:])
            sent12_t = cpool.tile([1, 384], dt.bfloat16, tag="s12")
            nc.sync.dma_start(out=sent12_t[:], in_=sent12[:])
            sent3_t = cpool.tile([1, 128], dt.bfloat16, tag="s3")
            nc.sync.dma_start(out=sent3_t[:], in_=sent3[:])
            bias_t = []
            for i, bsrc in enumerate((bias1, bias2, bias3)):
                bt = cpool.tile([P, HID], dt.float32, tag=f"b{i}")
                nc.sync.dma_start(out=bt[:], in_=bsrc[:])
                bias_t.append(bt)

            iota_row = cpool.tile([P, T * P], dt.bfloat16, tag="ior")
            nc.gpsimd.iota(iota_row[:], pattern=[[0, T], [1, P]],
                           channel_multiplier=0,
                           allow_small_or_imprecise_dtypes=True)
            iota_col = cpool.tile([P, 1], dt.float32, tag="ioc")
            nc.gpsimd.iota(iota_col[:], pattern=[[0, 1]], channel_multiplier=1,
                           allow_small_or_imprecise_dtypes=True)
            ones1 = cpool.tile([1, P], dt.bfloat16, tag="on1")
            nc.vector.memset(ones1[:], 1.0)
            ident = cpool.tile([P, P], dt.bfloat16, tag="idn")
            nc.gpsimd.memset(ident[:], 0.0)
            nc.gpsimd.affine_select(out=ident[:], in_=ident[:],
                                    compare_op=mybir.AluOpType.not_equal,
                                    fill=1.0, base=0, channel_multiplier=-1,
                                    pattern=[[1, P]])

            # ---- internal DRAM ----
            tabs = [
                dram.tile([NR, 384], dt.bfloat16, tag="tab1", name="tab1"),
                dram.tile([NR, 384], dt.bfloat16, tag="tab2", name="tab2"),
                dram.tile([NR, 128], dt.bfloat16, tag="tab3", name="tab3"),
            ]
            hTloc = [
                dram.tile([HID, NPC], dt.bfloat16, tag="h1l", name="h1l"),
                dram.tile([HID, NPC], dt.bfloat16, tag="h2l", name="h2l"),
            ]
            hTfull = [
                dram.tile([CORES, HID, NPC], dt.bfloat16, tag="h1f", name="h1f"),
                dram.tile([CORES, HID, NPC], dt.bfloat16, tag="h2f", name="h2f"),
            ]
            adstown = [
                dram.tile([BPC, P, H12], dt.bfloat16, tag="ad2", name="ad2"),
                dram.tile([BPC, P, 1], dt.bfloat16, tag="ad3", name="ad3"),
            ]

            waug_ts = [w1aug_t, w2aug_t, w3aug_t]
            sent_ts = [sent12_t, sent12_t, sent3_t]

            for L in range(3):
                c = cfgs[L]
                H, TCH, HWC, ASO, ADO, OC, F = (c['H'], c['TCH'], c['HWC'],
                                                c['ASO'], c['ADO'], c['OC'], c['F'])
                tab = tabs[L]

                # ---- produce table ----
                for g in range(NT):
                    if L == 0:
                        lx = sb3.tile([P, P], dt.bfloat16, tag="lx")
                        nc.sync.dma_start(out=lx[:], in_=xT[:, g * P:(g + 1) * P])
                        lhsT = lx[:, :]
                    else:
                        lx = sb3.tile([HID, P], dt.bfloat16, tag="lh")
                        src_core = g // BPC
                        col0 = (g % BPC) * P
                        nc.sync.dma_start(
                            out=lx[:], in_=hTfull[L - 1][src_core, :, col0:col0 + P])
                        lhsT = lx[:, :]
                    pps = ps.tile([P, OC], dt.float32, tag="pmain")
                    nc.tensor.matmul(pps[:], lhsT=lhsT, rhs=waug_ts[L][:F, :OC],
                                     start=True, stop=True)
                    ob = sb3.tile([P, OC], dt.bfloat16, tag="ob")
                    nc.vector.tensor_copy(out=ob[:], in_=pps[:])
                    nc.sync.dma_start(out=tab[1 + g * P:1 + (g + 1) * P, 0:OC],
                                      in_=ob[:])
                # sentinel rows
                nc.sync.dma_start(out=tab[SENT_LO:SENT_LO + 1, :], in_=sent_ts[L][:])
                nc.sync.dma_start(out=tab[SENT_HI:SENT_HI + 1, :], in_=sent_ts[L][:])

                # ---- gather + aggregate per dst block ----
                for b in range(BPC):
                    it = sb.tile([P, T * 8], dt.int16, tag="it")
                    nc.sync.dma_start(out=it[:], in_=idx[b])
                    dct = sb.tile([P, T], dt.bfloat16, tag="dct")
                    nc.sync.dma_start(out=dct[:], in_=dstc[b])
                    drt = sb.tile([1, T * P], dt.bfloat16, tag="drt")
                    nc.sync.dma_start(out=drt[:], in_=dstr[b])
                    adt = sb.tile([P, H], dt.bfloat16, tag="adt")
                    if L == 0:
                        nc.sync.dma_start(out=adt[:], in_=adst1own[b])
                    else:
                        nc.sync.dma_start(out=adt[:], in_=adstown[L - 1][b])

                    gt = sb.tile([P, T, TCH], dt.bfloat16, tag="g")
                    nc.gpsimd.dma_gather(
                        gt[:, 0:TA, :], tab[:, :], it[:, 0:TA * 8],
                        num_idxs=TA * P, num_idxs_reg=TA * P,
                        elem_size=TCH, single_packet=False,
                        queue_num=(2 * b) % 4)
                    nc.gpsimd.dma_gather(
                        gt[:, TA:T, :], tab[SPLIT:, :], it[:, TA * 8:T * 8],
                        num_idxs=TB * P, num_idxs_reg=TB * P,
                        elem_size=TCH, single_packet=False,
                        queue_num=(2 * b + 1) % 4)

                    # S01_ed[e, (t, d)] = (dst_local[e, t] == d)
                    s01ed = sb.tile([P, T * P], dt.bfloat16, tag="s01ed")
                    nc.vector.tensor_tensor(
                        out=s01ed[:].rearrange("p (t d) -> p t d", d=P),
                        in0=dct[:].to_broadcast([P, T, P]),
                        in1=iota_row[:].rearrange("p (t d) -> p t d", d=P),
                        op=mybir.AluOpType.is_equal)

                    # S01_de[d, (t, e)] = (dst_local[e, t] == d), via
                    # ones-matmul partition replication of the row layout.
                    s01de = sb.tile([P, T * P], dt.bfloat16, tag="s01de")
                    nchunks = (T * P + 511) // 512
                    for ch in range(nchunks):
                        lo = ch * 512
                        hi = min(T * P, lo + 512)
                        drep = ps.tile([P, 512], dt.float32, tag="paux")
                        nc.tensor.matmul(drep[:, :hi - lo], lhsT=ones1[:],
                                         rhs=drt[0:1, lo:hi], start=True, stop=True)
                        nc.vector.tensor_tensor(
                            out=s01de[:, lo:hi],
                            in0=drep[:, :hi - lo],
                            in1=iota_col[:].to_broadcast([P, hi - lo]),
                            op=mybir.AluOpType.is_equal)

                    # adst expanded per edge slot: [128e, H] per tile
                    padt = ps.tile([P, H * T], dt.float32, tag="padt")
                    for t in range(T):
                        nc.tensor.matmul(padt[:, H * t:H * (t + 1)],
                                         lhsT=s01de[:, t * P:(t + 1) * P],
                                         rhs=adt[:, 0:H], start=True, stop=True)

                    # w = exp(leaky_relu(asrc + adst))
                    epre = sb.tile([P, T * H], dt.float32, tag="epre")
                    nc.vector.tensor_tensor(
                        out=epre[:].rearrange("p (t h) -> p t h", h=H),
                        in0=gt[:, :, ASO:ASO + H],
                        in1=padt[:].rearrange("p (t h) -> p t h", h=H),
                        op=mybir.AluOpType.add)
                    wsc = sb.tile([P, T * H], dt.float32, tag="wsc")
                    nc.vector.tensor_scalar(out=wsc[:], in0=epre[:],
                                            scalar1=NEG_SLOPE, scalar2=None,
                                            op0=mybir.AluOpType.mult)
                    wlr = sb.tile([P, T * H], dt.float32, tag="wlr")
                    nc.vector.tensor_tensor(out=wlr[:], in0=epre[:], in1=wsc[:],
                                            op=mybir.AluOpType.max)
                    wt = sb.tile([P, T * H], dt.float32, tag="wt")
                    nc.scalar.activation(out=wt[:], in_=wlr[:],
                                         func=mybir.ActivationFunctionType.Exp)

                    # msg' = hw * w (broadcast over 64 ch), plus w columns
                    ms = sb.tile([P, T, HWC + H], dt.bfloat16, tag="ms")
                    nc.vector.tensor_tensor(
                        out=ms[:, :, 0:HWC].rearrange("p t (h c) -> p t h c", c=HID),
                        in0=gt[:, :, 0:HWC].rearrange("p t (h c) -> p t h c", c=HID),
                        in1=wt[:].rearrange("p (t h) -> p t h", h=H).to_broadcast([P, T, H, HID]),
                        op=mybir.AluOpType.mult)
                    nc.vector.tensor_copy(
                        out=ms[:, :, HWC:HWC + H],
                        in_=wt[:].rearrange("p (t h) -> p t h", h=H))

                    # aggregate: po[d, :] = sum_e S01[e, d] * ms[e, :]
                    po = ps.tile([P, HWC + H], dt.float32, tag="pmain")
                    for t in range(T):
                        nc.tensor.matmul(po[:], lhsT=s01ed[:, t * P:(t + 1) * P],
                                         rhs=ms[:, t, :],
                                         start=(t == 0), stop=(t == T - 1))

                    # epilogue
                    sreg = sb.tile([P, H], dt.float32, tag="sreg")
                    nc.vector.tensor_scalar(out=sreg[:], in0=po[:, HWC:HWC + H],
                                            scalar1=1e-9, scalar2=None,
                                            op0=mybir.AluOpType.add)
                    rre = sb.tile([P, H], dt.float32, tag="rre")
                    nc.vector.reciprocal(out=rre[:], in_=sreg[:])
                    if H > 1:
                        rr4 = sb.tile([P, H], dt.float32, tag="rr4")
                        nc.vector.tensor_scalar(out=rr4[:], in0=rre[:],
                                                scalar1=1.0 / H, scalar2=None,
                                                op0=mybir.AluOpType.mult)
                    else:
                        rr4 = rre
                    onrm = sb.tile([P, HWC], dt.float32, tag="onrm")
                    nc.vector.tensor_tensor(
                        out=onrm[:].rearrange("p (h c) -> p h c", c=HID),
                        in0=po[:, 0:HWC].rearrange("p (h c) -> p h c", c=HID),
                        in1=rr4[:].to_broadcast([P, H, HID]),
                        op=mybir.AluOpType.mult)
                    if H > 1:
                        t1 = sb.tile([P, HID], dt.float32, tag="t1")
                        nc.vector.tensor_tensor(out=t1[:], in0=onrm[:, 0:HID],
                                                in1=onrm[:, HID:2 * HID],
                                                op=mybir.AluOpType.add)
                        t2 = sb.tile([P, HID], dt.float32, tag="t2")
                        nc.vector.tensor_tensor(out=t2[:], in0=onrm[:, 2 * HID:3 * HID],
                                                in1=onrm[:, 3 * HID:4 * HID],
                                                op=mybir.AluOpType.add)
                        hsum = sb.tile([P, HID], dt.float32, tag="hsum")
                        nc.vector.tensor_tensor(out=hsum[:], in0=t1[:], in1=t2[:],
                                                op=mybir.AluOpType.add)
                    else:
                        hsum = onrm
                    hbias = sb.tile([P, HID], dt.float32, tag="hbias")
                    nc.vector.tensor_tensor(out=hbias[:], in0=hsum[:],
                                            in1=bias_t[L][:],
                                            op=mybir.AluOpType.add)
                    if L < 2:
                        # ELU = max(x,0) + exp(min(x,0)) - 1
                        emn = sb.tile([P, HID], dt.float32, tag="emn")
                        nc.vector.tensor_scalar(out=emn[:], in0=hbias[:],
                                                scalar1=0.0, scalar2=None,
                                                op0=mybir.AluOpType.min)
                        eex = sb.tile([P, HID], dt.float32, tag="eex")
                        nc.scalar.activation(out=eex[:], in_=emn[:],
                                             func=mybir.ActivationFunctionType.Exp)
                        emx = sb.tile([P, HID], dt.float32, tag="emx")
                        nc.vector.tensor_scalar(out=emx[:], in0=hbias[:],
                                                scalar1=0.0, scalar2=None,
                                                op0=mybir.AluOpType.max)
                        esum = sb.tile([P, HID], dt.float32, tag="esum")
                        nc.vector.tensor_tensor(out=esum[:], in0=eex[:], in1=emx[:],
                                                op=mybir.AluOpType.add)
                        hb16 = sb.tile([P, HID], dt.bfloat16, tag="hb16")
                        nc.vector.tensor_scalar(out=hb16[:], in0=esum[:],
                                                scalar1=-1.0, scalar2=None,
                                                op0=mybir.AluOpType.add)
                        # transpose h block -> [64, 128] for next produce
                        pt = ps.tile([HID, P], dt.bfloat16, tag="paux")
                        nc.tensor.transpose(out=pt[:], in_=hb16[:], identity=ident[:])
                        ht = sb.tile([HID, P], dt.bfloat16, tag="ht")
                        nc.vector.tensor_copy(out=ht[:], in_=pt[:])
                        nc.sync.dma_start(out=hTloc[L][:, b * P:(b + 1) * P], in_=ht[:])
                        # adst for next layer's own nodes
                        Hn = cfgs[L + 1]['H']
                        ADOn = cfgs[L + 1]['ADO']
                        pan = ps.tile([P, Hn], dt.float32, tag="padt")
                        nc.tensor.matmul(pan[:], lhsT=ht[:],
                                         rhs=waug_ts[L + 1][:HID, ADOn:ADOn + Hn],
                                         start=True, stop=True)
                        adn = sb.tile([P, Hn], dt.bfloat16, tag="adn")
                        nc.vector.tensor_copy(out=adn[:], in_=pan[:])
                        nc.sync.dma_start(out=adstown[L][b], in_=adn[:])
                    else:
                        nc.sync.dma_start(out=out3[b * P:(b + 1) * P, :], in_=hbias[:])

                if L < 2:
                    nc.gpsimd.collective_compute(
                        "AllGather", mybir.AluOpType.bypass,
                        replica_groups=[list(range(CORES))],
                        ins=[hTloc[L].opt()],
                        outs=[hTfull[L].opt()])

    nc.compile()
    return nc


def _fold(W, a, heads):
    return np.einsum('fhc,hc->fh', W.reshape(W.shape[0], heads, HID), a)


def preprocess(x, edge_index, W1, a1_src, a1_dst, b1, W2, a2_src, a2_dst, b2,
               W3, a3_src, a3_dst, b3):
    """Build per-core in_maps + (TA, TB)."""
    x = np.asarray(x, np.float32)
    ei = np.asarray(edge_index).astype(np.int64)
    loop = np.arange(N, dtype=np.int64)
    src = np.concatenate([ei[0], loop])
    dst = np.concatenate([ei[1], loop])

    gblk = dst // P  # global block id, 0..391
    order = np.argsort(gblk, kind='stable')
    src_s = src[order]
    dst_s = dst[order]
    bounds = np.searchsorted(gblk[order], np.arange(CORES * BPC + 1))

    rows = src_s + 1             # table row of the source
    isA = rows < SPLIT
    dl = (dst_s % P).astype(np.int16)

    # per-block A/B lists
    blkA_idx, blkB_idx, blkA_dl, blkB_dl = [], [], [], []
    nAmax = nBmax = 0
    for gb in range(CORES * BPC):
        lo, hi = bounds[gb], bounds[gb + 1]
        a = isA[lo:hi]
        rA = rows[lo:hi][a]
        rB = rows[lo:hi][~a] - SPLIT
        blkA_idx.append(rA.astype(np.int16))
        blkB_idx.append(rB.astype(np.int16))
        blkA_dl.append(dl[lo:hi][a])
        blkB_dl.append(dl[lo:hi][~a])
        nAmax = max(nAmax, len(rA))
        nBmax = max(nBmax, len(rB))
    TA = max(1, -(-nAmax // P))
    TB = max(1, -(-nBmax // P))
    T = TA + TB

    sentB = SENT_HI - SPLIT
    idx_all = np.zeros((CORES, BPC, P, T * 8), np.int16)
    dstc_all = np.full((CORES, BPC, P, T), 127, BF16)
    dstr_all = np.full((CORES, BPC, 1, T * P), 127, BF16)
    for gb in range(CORES * BPC):
        cc, b = divmod(gb, BPC)
        ia = np.full(TA * P, SENT_LO, np.int16)
        ia[:len(blkA_idx[gb])] = blkA_idx[gb]
        ib = np.full(TB * P, sentB, np.int16)
        ib[:len(blkB_idx[gb])] = blkB_idx[gb]
        flat = np.concatenate([ia, ib])
        # wrap: position i -> [i % 16, i // 16], per gather range
        wa = ia.reshape(TA * 8, 16).T
        wb = ib.reshape(TB * 8, 16).T
        w = np.concatenate([wa, wb], axis=1)        # [16, T*8]
        idx_all[cc, b] = np.tile(w, (8, 1))
        da = np.full(TA * P, 127, np.int16)
        da[:len(blkA_dl[gb])] = blkA_dl[gb]
        db = np.full(TB * P, 127, np.int16)
        db[:len(blkB_dl[gb])] = blkB_dl[gb]
        dflat = np.concatenate([da, db])            # [T*P] position-major
        dstc_all[cc, b] = dflat.reshape(T, P).T.astype(BF16)
        dstr_all[cc, b, 0] = dflat.astype(BF16)

    # weights
    def baug(W, asrc, adst, heads, tch):
        a = np.concatenate([W, _fold(W, asrc, heads), _fold(W, adst, heads)], axis=1)
        out = np.zeros((W.shape[0], tch), np.float32)
        out[:, :a.shape[1]] = a
        return out.astype(BF16)
    w1a = baug(np.asarray(W1, np.float32), np.asarray(a1_src, np.float32),
               np.asarray(a1_dst, np.float32), H12, 384)
    w2a = baug(np.asarray(W2, np.float32), np.asarray(a2_src, np.float32),
               np.asarray(a2_dst, np.float32), H12, 384)
    w3a = baug(np.asarray(W3, np.float32), np.asarray(a3_src, np.float32),
               np.asarray(a3_dst, np.float32), 1, 128)

    s12 = np.zeros((1, 384), BF16)
    s12[0, H12 * HID:H12 * HID + H12] = ASRC_SENT
    s3 = np.zeros((1, 128), BF16)
    s3[0, HID] = ASRC_SENT

    xp = np.zeros((NP, F_IN), np.float32)
    xp[:N] = x
    xTb = np.ascontiguousarray(xp.T).astype(BF16)

    # host analog of layer-1 adst for each core's own nodes
    ad1 = (xp.astype(BF16).astype(np.float32)
           @ _fold(np.asarray(W1, np.float32), np.asarray(a1_dst, np.float32),
                   H12).astype(BF16).astype(np.float32)).astype(BF16)

    b1r = np.tile(np.asarray(b1, np.float32)[None, :], (P, 1))
    b2r = np.tile(np.asarray(b2, np.float32)[None, :], (P, 1))
    b3r = np.tile(np.asarray(b3, np.float32)[None, :], (P, 1))

    in_maps = []
    for cc in range(CORES):
        in_maps.append({
            "xT": xTb, "w1aug": w1a, "w2aug": w2a, "w3aug": w3a,
            "sent12": s12, "sent3": s3,
            "bias1": b1r, "bias2": b2r, "bias3": b3r,
            "idx": idx_all[cc], "dstc": dstc_all[cc], "dstr": dstr_all[cc],
            "adst1own": ad1[cc * NPC:(cc + 1) * NPC].reshape(BPC, P, H12),
        })
    return in_maps, TA, TB


_CACHE = {}


def kernel(x, edge_index, batch,
           W1, a1_src, a1_dst, b1,
           W2, a2_src, a2_dst, b2,
           W3, a3_src, a3_dst, b3,
           fc1_W, fc1_b, fc2_W, fc2_b, fc3_W, fc3_b):
    global LAST_EXEC_NS
    trace = os.environ.get("GAT_TRACE", "") == "1"
    if trace:
        _install_ntff_shim()

    in_maps, TA, TB = preprocess(x, edge_index,
                                 W1, a1_src, a1_dst, b1,
                                 W2, a2_src, a2_dst, b2,
                                 W3, a3_src, a3_dst, b3)
    key = (TA, TB)
    if key not in _CACHE:
        _CACHE[key] = build_program(TA, TB)
    nc = _CACHE[key]

    res = run_bass_kernel_spmd(nc, in_maps, list(range(CORES)), trace=trace)
    LAST_EXEC_NS = res.exec_time_ns

    h3 = np.concatenate([np.asarray(res.results[cc]["out3"]) for cc in range(CORES)],
                        axis=0)[:N]

    batch = np.asarray(batch).astype(np.int64)
    counts = np.bincount(batch, minlength=G).astype(np.float32)
    pooled = np.zeros((G, HID), np.float32)
    np.add.at(pooled, batch, h3)
    pooled = pooled / np.maximum(counts, 1.0)[:, None]
    z = np.maximum(pooled @ np.asarray(fc1_W, np.float32) + np.asarray(fc1_b, np.float32), 0.0)
    z = np.maximum(z @ np.asarray(fc2_W, np.float32) + np.asarray(fc2_b, np.float32), 0.0)
    return (z @ np.asarray(fc3_W, np.float32) + np.asarray(fc3_b, np.float32)).astype(np.float32)
